# revision 20
# baseline (speedup 1.0000x reference)
"""BERT-with-RoPE attention layer on 8 Trainium2 NeuronCores.

Sharding: core c handles (batch b = c//2, sequence-half hf = c%2).
Each core computes k/v for its batch's full 2048 rows (k/v work duplicated
2x across the pair) and q + attention + out-projection for its own 1024
query rows, so the 8 output shards are disjoint and the host gather is a
pure concatenation (no collectives).

v7 — exp-stream-first schedule. The scalar engine's exp stream (256 blocks
x ~1.1us) is the scarcest resource after the PE; everything else is
arranged so it starts early and never stalls:
  - v-projection is split: heads 0-7 run before attention (so the first
    scores fire ~60us earlier), heads 8-15 ride the attention slack.
  - qk-projection spans are emitted in half-span chunks (8 matmuls) spread
    across the block loop instead of 16-matmul bursts.
  - the qkv bias is folded into scalar_tensor_tensor rope combines on the
    vector/pool engines reading the span PSUM directly; the scalar engine
    does nothing but exp during attention.
  - the NeoX half-swap is a SBUF->SBUF DMA partition remap (sign folded
    into the sin table) instead of a Pswap matmul.
  - out-projection accumulates in the span PSUM pool; its hf=0 half rides
    the last attention window, bias applied on the vector engine.
PSUM budget: scores 2x[128,2,512] (4 banks) + ctx cE/cO (2) + acc pool
2x[128,512] (2, shared by v-proj, span and out-proj accumulation) = 8.
"""

import os
import numpy as np

B, S, H = 4, 2048, 1024
NH, DH = 16, 64
HALF = DH // 2
SQ = S // 2  # query rows per core
KC = H // 128  # hidden contraction chunks
ROPE_BASE = 10000.0
N_CORES = 8

_nc_cache = None
last_results = None


def _build_nc():
    import concourse.bacc as bacc
    import concourse.mybir as mybir
    from concourse.tile import TileContext

    f32 = mybir.dt.float32
    bf16 = mybir.dt.bfloat16
    Exp = mybir.ActivationFunctionType.Exp
    Copy = mybir.ActivationFunctionType.Copy
    MUL = mybir.AluOpType.mult
    ADD = mybir.AluOpType.add

    nc = bacc.Bacc(None, target_bir_lowering=False)

    xT_d = nc.dram_tensor("xT", [KC, 128, S], bf16, kind="ExternalInput")
    wqk_d = nc.dram_tensor("wqk", [16, 128, KC, 128], bf16, kind="ExternalInput")
    wv_d = nc.dram_tensor("wv", [128, KC, H], bf16, kind="ExternalInput")
    wout_d = nc.dram_tensor("wout", [128, 8, KC, 128], bf16, kind="ExternalInput")
    cosk_d = nc.dram_tensor("cosk", [128, S], f32, kind="ExternalInput")
    sink_d = nc.dram_tensor("sink", [128, S], f32, kind="ExternalInput")
    bqk_d = nc.dram_tensor("bqk", [128, 16], f32, kind="ExternalInput")
    boutp_d = nc.dram_tensor("boutp", [128, 8], f32, kind="ExternalInput")
    out_d = nc.dram_tensor("outT", [8, 128, SQ], f32, kind="ExternalOutput")
    debug = bool(int(os.environ.get("KERNEL_DEBUG", "0") or "0"))
    if debug:
        dq_d = nc.dram_tensor("dq", [128, 8, 2, 512], bf16, kind="ExternalOutput")
        dk_d = nc.dram_tensor("dk", [128, 8, 4, 512], bf16, kind="ExternalOutput")
        dv_d = nc.dram_tensor("dv", [128, 16, NH, 72], bf16, kind="ExternalOutput")
        dctx_d = nc.dram_tensor("dctx", [128, KC, SQ], bf16, kind="ExternalOutput")

    with TileContext(nc) as tc:
        if True:
            const = tc.alloc_tile_pool(name="const", bufs=1)
            persist = tc.alloc_tile_pool(name="persist", bufs=1)
            xTp = tc.alloc_tile_pool(name="xTp", bufs=1)
            mapp = tc.alloc_tile_pool(name="mapp", bufs=1)
            wqkp = tc.alloc_tile_pool(name="wqkp", bufs=6)
            ttp = tc.alloc_tile_pool(name="ttp", bufs=2)
            ttsp = tc.alloc_tile_pool(name="ttsp", bufs=2)
            ccp = tc.alloc_tile_pool(name="ccp", bufs=2)
            qp = tc.alloc_tile_pool(name="qp", bufs=3)
            kp = tc.alloc_tile_pool(name="kp", bufs=3)
            woutp = tc.alloc_tile_pool(name="woutp", bufs=1)
            ctxp = tc.alloc_tile_pool(name="ctxp", bufs=1)
            expp = tc.alloc_tile_pool(name="expp", bufs=2)
            scrp = tc.alloc_tile_pool(name="scrp", bufs=2)
            obp = tc.alloc_tile_pool(name="obp", bufs=2)
            bqk_sb = const.tile([128, 16], f32)
            boutp_sb = const.tile([128, 8], f32)

            # v resident in SBUF: [s2_in_blk, s2_blk, head, dcol+ones]
            v_sb = persist.tile([128, 16, NH, 72], bf16)
            ctxT = ctxp.tile([128, KC, SQ], bf16)
            cosk_sb = mapp.tile([128, 4, 512], f32)
            sink_sb = mapp.tile([128, 4, 512], f32)
            wout_sb = woutp.tile([128, 8, KC, 128], bf16, tag="wo", name="wout_sb")

            # ---------------- input DMA schedule -------------------------
            nc.gpsimd.memset(v_sb[:, :, :, DH : DH + 1], 1.0)
            xT_sb = [
                xTp.tile([128, S], bf16, tag=f"x{c}", name=f"xc{c}")
                for c in range(KC)
            ]
            wvp = tc.alloc_tile_pool(name="wvp", bufs=1)
            wvt = wvp.tile([128, KC, H], bf16, tag="wv", name="wvt")

            wt_tiles = {}

            def _wt_load(oc):
                wt = wqkp.tile([128, KC, 128], bf16, tag="w", name="wt")
                for h in range(2):
                    nc.sync.dma_start(
                        wt[:, h * 4 : (h + 1) * 4, :],
                        wqk_d[oc, :, h * 4 : (h + 1) * 4, :],
                    )
                wt_tiles[oc] = wt

            # wv chunk 0 / head-half 0 gates the very first matmul; pair-0
            # span weights + rope quarter 0 unblock the prologue spans
            nc.sync.dma_start(wvt[:, 0, 0:512], wv_d[:, 0, 0:512])
            nc.sync.dma_start(bqk_sb[:, :], bqk_d[:, :])
            _wt_load(0)
            _wt_load(8)
            nc.sync.dma_start(sink_sb[:, 0, :], sink_d[:, 0:512])
            nc.sync.dma_start(cosk_sb[:, 0, :], cosk_d[:, 0:512])
            for c in range(KC):
                for h in range(4):
                    nc.sync.dma_start(
                        xT_sb[c][:, h * 512 : (h + 1) * 512],
                        xT_d[c, :, h * 512 : (h + 1) * 512],
                    )
                if c > 0:
                    nc.sync.dma_start(wvt[:, c, 0:512], wv_d[:, c, 0:512])
            _wt_load(1)
            _wt_load(9)
            for h in range(1, 4):
                nc.sync.dma_start(sink_sb[:, h, :], sink_d[:, h * 512 : (h + 1) * 512])
                nc.sync.dma_start(cosk_sb[:, h, :], cosk_d[:, h * 512 : (h + 1) * 512])
            for c in range(KC):
                nc.sync.dma_start(wvt[:, c, 512:1024], wv_d[:, c, 512:1024])

            # ---------------- shared PSUM pools --------------------------
            # scores: 2 x [128,2,512] = 4 banks; ctx: 2 x [128,512] = 2;
            # acc (v-proj / spans / out-proj): 2 x [128,512] = 2.
            if True:
                scp = tc.alloc_tile_pool(name="scp", bufs=2, space="PSUM")
                psCtx = tc.alloc_tile_pool(name="psCtx", bufs=1, space="PSUM")
                accp = tc.alloc_tile_pool(name="accp", bufs=2, space="PSUM")
                q_tiles, k_tiles = {}, {}

                # ---------- v-projection (per sb, per head-half) ----------
                def _v_chunk(sb, hv, eng):
                    ps = accp.tile([128, 512], f32, tag="acc", name="acc")
                    for c in range(KC):
                        nc.tensor.matmul(
                            ps[:, :],
                            xT_sb[c][:, sb * 128 : (sb + 1) * 128],
                            wvt[:, c, hv * 512 : (hv + 1) * 512],
                            start=(c == 0), stop=(c == KC - 1),
                        )
                    dst = v_sb[:, sb, hv * 8 : (hv + 1) * 8, 0:DH]
                    src = ps.rearrange("p (h d) -> p h d", h=8)
                    if eng == "act":
                        nc.scalar.activation(dst, src, Copy)
                    else:
                        nc.vector.tensor_copy(dst, src)

                # ---------- qk span halves ---------------------------------
                # each span (oc, sp) covers 1024 rows = two 512 halves; a
                # half emits 8 matmuls into a 1-bank accumulator, then the
                # bias+sin and bias+cos combines on the vector engine (the
                # only non-scalar engine that reads PSUM). After the second
                # half, the NeoX 32-row swap runs as 4 slice DMAs (grouped
                # APs miscompile; issuance split across vector+pool engines
                # to keep the sync queue clear) and one pool-engine add
                # produces the final bf16 q/k quarter-pair.
                span_st = {}

                def _hspan(oc, sp, hv, dst2):
                    ps = accp.tile([128, 512], f32, tag="acc", name="acc")
                    for c in range(KC):
                        nc.tensor.matmul(
                            ps[:, :],
                            wt_tiles[oc][:, c, :],
                            xT_sb[c][:, sp * SQ + hv * 512 : sp * SQ + (hv + 1) * 512],
                            start=(c == 0), stop=(c == KC - 1),
                        )
                    if hv == 0:
                        tt = ttp.tile([128, 2, 512], bf16, tag="tt", name="tt")
                        cc = ccp.tile([128, 2, 512], f32, tag="cc", name="cc")
                        span_st[(oc, sp)] = (tt, cc)
                    else:
                        tt, cc = span_st.pop((oc, sp))
                    mp = 2 * sp + hv
                    nc.vector.scalar_tensor_tensor(
                        tt[:, hv, :], ps[:, :], bqk_sb[:, oc : oc + 1],
                        sink_sb[:, mp, :], ADD, MUL,
                    )
                    nc.vector.scalar_tensor_tensor(
                        cc[:, hv, :], ps[:, :], bqk_sb[:, oc : oc + 1],
                        cosk_sb[:, mp, :], ADD, MUL,
                    )
                    if hv == 1:
                        tts = ttsp.tile([128, 2, 512], bf16, tag="tts", name="tts")
                        for g in range(4):
                            sg = g ^ 1
                            nc.gpsimd.dma_start(
                                tts[g * 32 : (g + 1) * 32, :, :],
                                tt[sg * 32 : (sg + 1) * 32, :, :],
                            )
                        nc.gpsimd.tensor_tensor(
                            dst2, cc[:, :, :], tts[:, :, :], ADD
                        )

                # six chunks per pair; k first so pair p's k tiles are
                # complete before its first scores block needs all of them
                def _pair_hspan(p, which):
                    if p > 7:
                        return
                    if which == 0:
                        k_tiles[p] = kp.tile([128, 4, 512], bf16, tag="k", name="kt")
                        q_tiles[p] = qp.tile([128, 2, 512], bf16, tag="q", name="qt")
                        _hspan(8 + p, 0, 0, None)
                    elif which == 1:
                        _hspan(8 + p, 0, 1, k_tiles[p][:, 0:2, :])
                    elif which == 2:
                        _hspan(8 + p, 1, 0, None)
                    elif which == 3:
                        _hspan(8 + p, 1, 1, k_tiles[p][:, 2:4, :])
                    elif which == 4:
                        _hspan(p, 0, 0, None)
                    else:
                        _hspan(p, 0, 1, q_tiles[p][:, 0:2, :])
                        if p + 2 <= 7:
                            _wt_load(p + 2)
                            _wt_load(8 + p + 2)

                # ---------- out-projection chunk (per hf, hb) -------------
                def _d_chunk(hf, hb):
                    s1 = slice(hf * 512, (hf + 1) * 512)
                    ps = accp.tile([128, 512], f32, tag="acc", name="acc")
                    for c in range(KC):
                        nc.tensor.matmul(
                            ps[:, :],
                            wout_sb[:, hb, c, :],
                            ctxT[:, c, s1],
                            start=(c == 0), stop=(c == KC - 1),
                        )
                    ob = obp.tile([128, 512], f32, tag="ob", name="ob")
                    nc.vector.tensor_scalar(
                        ob[:, :], ps[:, :], boutp_sb[:, hb : hb + 1], None, ADD
                    )
                    for h in range(2):
                        nc.sync.dma_start(
                            out_d[hb, :, hf * 512 + h * 256 : hf * 512 + (h + 1) * 256],
                            ob[:, h * 256 : (h + 1) * 256],
                        )

                # ---------------- prologue -------------------------------
                # v heads 0-7 for all 16 seq blocks, interleaved with the
                # q/k half-spans for pairs 0 and 1
                prolog_spans = [(p, w) for p in range(2) for w in range(6)]
                for sb in range(16):
                    _v_chunk(sb, 0, "act")
                    if sb % 4 == 1 and prolog_spans:
                        for _ in range(3):
                            if prolog_spans:
                                _pair_hspan(*prolog_spans.pop(0))
                while prolog_spans:
                    _pair_hspan(*prolog_spans.pop(0))

                # bulk weights for the tail phases, after the hot prologue
                nc.sync.dma_start(boutp_sb[:, :], boutp_d[:, :])
                for hb in range(8):
                    nc.sync.dma_start(wout_sb[:, hb, :, :], wout_d[:, hb, :, :])

                # ---------------- aux work queue -------------------------
                # per (pr, hf) window: list of closures run at quiet blocks
                def _build_aux():
                    aux = {}
                    for pr in range(8):
                        for hf in range(2):
                            q = []
                            # spans for pair pr+2 (6 half-spans, 2 windows)
                            base = 3 * hf
                            for w in (base, base + 1, base + 2):
                                q.append(lambda p=pr + 2, w=w: _pair_hspan(p, w))
                            # v heads 8-15: 16 chunks over pairs 0-3 windows
                            if pr < 4:
                                for i in range(2):
                                    sb = 4 * pr + 2 * hf + i
                                    q.append(lambda sb=sb: _v_chunk(sb, 1, "gps"))
                            # out-projection hf=0 rides the last window
                            if pr == 7 and hf == 1:
                                for hb in range(8):
                                    q.append(lambda hb=hb: _d_chunk(0, hb))
                            aux[(pr, hf)] = q
                    return aux

                aux = _build_aux()

                # ---------------- attention loop -------------------------
                for pr in range(8):
                    for hf in range(2):
                        queue = aux[(pr, hf)]
                        s1 = slice(hf * 512, (hf + 1) * 512)
                        cE = psCtx.tile([128, 512], f32, tag="ctxe", name="cE")
                        cO = psCtx.tile([128, 512], f32, tag="ctxo", name="cO")

                        def _ctx(blk, et):
                            st, sp_ = (blk == 0), (blk == 15)
                            nc.tensor.matmul(
                                cE[0 : DH + 1, :],
                                v_sb[:, blk, 2 * pr, 0 : DH + 1], et[:, 0, :],
                                start=st, stop=sp_,
                            )
                            nc.tensor.matmul(
                                cO[0 : DH + 1, :],
                                v_sb[:, blk, 2 * pr + 1, 0 : DH + 1], et[:, 1, :],
                                start=st, stop=sp_,
                            )

                        # software-pipelined: ctx(blk-1) is emitted after
                        # scores(blk), giving exp(blk-1) a full block period
                        # before the PE needs its result
                        prev_et = None
                        for blk in range(16):
                            sc = scp.tile([128, 2, 512], f32, tag="sc", name="sc")
                            for par in range(2):
                                rs = par * 64
                                nc.tensor.matmul(
                                    sc[:, par, :],
                                    k_tiles[pr][
                                        rs : rs + 64, blk // 4,
                                        (blk % 4) * 128 : (blk % 4) * 128 + 128,
                                    ],
                                    q_tiles[pr][rs : rs + 64, hf, :],
                                    start=True, stop=True,
                                )
                            et = expp.tile([128, 2, 512], bf16, tag="et", name="et")
                            nc.scalar.activation(
                                et[:, :, :], sc[:, :, :], Exp, scale=0.125
                            )
                            if prev_et is not None:
                                _ctx(blk - 1, prev_et)
                            prev_et = et
                            if blk in (1, 4, 7, 10, 13) and queue:
                                queue.pop(0)()
                        _ctx(15, prev_et)
                        while queue:
                            queue.pop(0)()
                        # epilogue: normalize ctx rows 0..63 by sums row 64.
                        # ct is staged to SBUF in one copy so the single-
                        # buffered ctx PSUM frees before the next window's
                        # first accumulation needs it; everything after
                        # works from SBUF. partition_broadcast reads tensor
                        # partition 0, so the sums row routes through
                        # partition 0 via a pool-engine DMA.
                        for par, ct in ((0, cE), (1, cO)):
                            cts = scrp.tile([65, 512], f32, tag="cts", name="cts")
                            nc.vector.tensor_copy(cts[:, :], ct[0:65, :])
                            scr2 = scrp.tile([1, 512], f32, tag="scr2", name="scr2")
                            nc.gpsimd.dma_start(scr2[0:1, :], cts[64:65, :])
                            bcs = scrp.tile([128, 512], f32, tag="bcs", name="bcs")
                            nc.gpsimd.partition_broadcast(bcs[0:64, :], scr2[0:1, :])
                            bc = bcs
                            nc.vector.reciprocal_approx_fast(bc[0:64, :], bcs[0:64, :])
                            if par == 0:
                                nc.vector.tensor_tensor(
                                    ctxT[0:64, pr, s1], cts[0:64, :], bc[0:64, :], MUL
                                )
                            else:
                                tmp = scrp.tile([64, 512], bf16, tag="tmp", name="tmp")
                                nc.vector.tensor_tensor(
                                    tmp[:, :], cts[0:64, :], bc[0:64, :], MUL
                                )
                                nc.sync.dma_start(ctxT[64:128, pr, s1], tmp[:, :])

                # ---------------- out-projection hf=1 tail ----------------
                for hb in range(8):
                    _d_chunk(1, hb)

            if debug:
                for p in range(8):
                    nc.sync.dma_start(dq_d[:, p, :, :], q_tiles[p][:, :, :])
                    nc.sync.dma_start(dk_d[:, p, :, :], k_tiles[p][:, :, :])
                nc.sync.dma_start(dv_d[:, :, :, :], v_sb[:, :, :, :])
                nc.sync.dma_start(dctx_d[:, :, :], ctxT[:, :, :])

            # release in LIFO order per space (SBUF stack, then PSUM stack)
            for pool in (accp, psCtx, scp, wvp, obp, scrp, expp, ctxp,
                         woutp, kp, qp, ccp, ttsp, ttp, wqkp, mapp, xTp,
                         persist, const):
                pool.release()

    nc.finalize()
    return nc


def _host_prep(positions, hidden_states, Wqkv, bqkv, Wout, bout):
    import ml_dtypes

    bf16 = ml_dtypes.bfloat16
    positions = np.asarray(positions)
    hidden_states = np.asarray(hidden_states, dtype=np.float32)
    Wqkv = np.asarray(Wqkv, dtype=np.float32)
    bqkv = np.asarray(bqkv, dtype=np.float32)
    Wout = np.asarray(Wout, dtype=np.float32)
    bout = np.asarray(bout, dtype=np.float32)

    # wqk[oc][p][c][128]: per-oc weight tile with 2KB-contiguous lines
    wqk = np.ascontiguousarray(
        Wqkv[:, : 2 * H].reshape(KC, 128, 16, 128).transpose(2, 1, 0, 3)
    ).astype(bf16)
    # wv[p][c][H]: single-tile load, partition = row within chunk
    wv = np.ascontiguousarray(
        Wqkv[:, 2 * H :].reshape(KC, 128, H).transpose(1, 0, 2)
    ).astype(bf16)
    # wout[p][hb][c][128]
    wout_t = np.ascontiguousarray(
        Wout.reshape(KC, 128, 8, 128).transpose(1, 2, 0, 3)
    ).astype(bf16)
    bqk = np.ascontiguousarray(bqkv[: 2 * H].reshape(16, 128).T)
    boutp_full = bout.astype(np.float64) + bqkv[2 * H :].astype(
        np.float64
    ) @ Wout.astype(np.float64)
    boutp = np.ascontiguousarray(boutp_full.astype(np.float32).reshape(8, 128).T)

    inv_freq = 1.0 / (ROPE_BASE ** (np.arange(HALF, dtype=np.float64) / HALF))
    rowmap = np.arange(128) % HALF
    # sign folded into sin: row p feeds destination swap(p); dest first half
    # gets -x2*sin (so rows 32:64 within each head carry -), dest second
    # half gets +x1*sin (rows 0:32 carry +)
    sinsign = np.where((np.arange(128) % DH) < HALF, 1.0, -1.0)[:, None]

    in_maps = []
    for c in range(N_CORES):
        b, hf = c // 2, c % 2
        perm = np.concatenate(
            [np.arange(hf * SQ, (hf + 1) * SQ), np.arange((1 - hf) * SQ, (2 - hf) * SQ)]
        )
        x_perm = hidden_states[b][perm]
        xT = np.ascontiguousarray(x_perm.T).reshape(KC, 128, S).astype(bf16)
        pos = positions[perm].astype(np.float64)
        freqs = pos[:, None] * inv_freq[None, :]  # [S, HALF]
        cosk = np.ascontiguousarray(np.cos(freqs).astype(np.float32)[:, rowmap].T)
        sink = np.ascontiguousarray(
            (np.sin(freqs).astype(np.float32)[:, rowmap].T * sinsign).astype(np.float32)
        )
        in_maps.append(
            {
                "xT": xT, "wqk": wqk, "wv": wv, "wout": wout_t,
                "cosk": cosk, "sink": sink,
                "bqk": bqk, "boutp": boutp,
            }
        )
    return in_maps


def kernel(positions, hidden_states, Wqkv, bqkv, Wout, bout):
    global _nc_cache, last_results
    from concourse import bass_utils

    if _nc_cache is None:
        _nc_cache = _build_nc()
    nc = _nc_cache

    in_maps = _host_prep(positions, hidden_states, Wqkv, bqkv, Wout, bout)
    res = bass_utils.run_bass_kernel_spmd(
        nc, in_maps, core_ids=list(range(N_CORES)),
        trace=bool(int(os.environ.get("KERNEL_TRACE", "0") or "0")),
    )
    last_results = res

    out = np.empty((B, S, H), dtype=np.float32)
    for c in range(N_CORES):
        b, hf = c // 2, c % 2
        outT = np.asarray(res.results[c]["outT"]).reshape(H, SQ)
        out[b, hf * SQ : (hf + 1) * SQ, :] = outT.T
    return out
